# revision 52
# baseline (speedup 1.0000x reference)
"""Trainium2 Bass kernel for nn_CenterLoss (retrieval_knn).

reference semantics (per batch b):
    dist[n, m] = ||pred[b, n] - gt[b, m]||^2           (N=4096, M=512)
    dist1[n] = min_m dist ; dist2[m] = min_n dist
    loss = sum(dist1*obj)/(sum(obj)+1e-6) + sum(dist2*mask)/(sum(mask)+1e-6)

Strategy: data-parallel over batch (16 batches -> 8 cores, 2 each). On each
core, per batch, the PE builds the NEGATED distance matrix T = -dist via a
K=20 augmented matmul (bf16 hi/lo split reproduces fp32 dots to ~2^-18):
    T[i, j] = sum_k pa[k, i] * ga[k, j]
Since K=20 <= 32, four pred-tiles' matmuls are packed onto the PE at once
with 32-row tile_position groups (pa rows at partitions 32r..32r+19, ga
replicated in the same partition bands), quadrupling PE throughput.

The reduction pipeline is engineered around two hard TRN2 facts: (1) matmul
PSUM output must be fp32, and only ACT reads PSUM at line rate, so the
minimal egress is one ACT fp32->fp16 copy of every element (~29us/core,
and ACT runs it back-to-back with zero gaps); (2) DVE tensor_tensor max
runs at 2x only on f16 SBUF operands. Everything else is shaved so DVE
stays just under ACT (the measured multi-engine optimum):
  - obj rows are permuted to the front on the host; row-max trees (dist1)
    run on only T_OBJ=18 of 32 tiles (objectness_label is 0/1, so only
    ~2048 rows need dist1; capacity asserted host-side).
  - trees stop at 64-wide partials (3 TT levels); the host finishes the max.
  - the column path (dist2) folds pack-pair maxes into a 2-plane macc2
    accumulator over the first 3 groups; the last group's f16 tiles ship
    raw straight off the ACT copy, so NO engine op chains after the final
    PSUM egress — the kernel-exit critical path is just the last two DMAs
    (split across both HWDGE rings) and their DRAM completion.
  - DVE siphons ~5% of the PSUM egress (2 tiles of batch 0's pack 1 during
    the fill, 1 tile at the batch boundary) — the computed ACT/DVE balance
    point. Readers of one PSUM tile are chained in emission order, so the
    ACT copy is always emitted before the DVE cast of the same pack.
  - partials (d1p row partials, macc2, raw pair-maxes) DMA to DRAM; the
    host does the cross-partition max and both masked sums in fp64. No
    on-chip transposes, masked sums, or aux inputs at all.
  - ga+pa share one DRAM tensor (single head-DMA issue on the fill path);
    out-DMAs are spread across both HWDGE rings with the final pair-max
    alone on the SP ring (DRAM completion latency dominates the tail).
Teardown: the walrus single-wait limit forces the end-of-kernel drain to be
split into single-wait NOPs; these are round-robined across all five
engines (a serial GpSimd chain costs ~6us) with a join semaphore that
GpSimd waits on before clearing semaphores.
"""

import numpy as np

B, N, M = 16, 4096, 512
N_CORES = 8
B_LOC = B // N_CORES        # batches per core
NT = N // 128               # pred tiles per batch (32)
PACK = 4                    # pred tiles packed per PE pass (32-row groups)
NP = NT // PACK             # packs per batch (8)
NG = NP // 2                # x8 groups of 8 tiles (4)
T_OBJ = 18                  # row-tree tile capacity (obj!=0 rows first)
L3W = 256                   # shipped row-partial width per tile (1 TT level)

_PROGRAM_CACHE = {}


def _install_walrus_ctrl_wait_workaround():
    """The installed walrus rejects multi-wait CTRL (Drain) instructions
    ("Too many sync wait commands"). Split the TileContext end-of-kernel
    drain's sem waits onto individual NOPs (one wait each), round-robined
    across all five engines so they retire in parallel; a join semaphore
    orders GpSimd's semaphore clears after every wait."""
    import concourse.tile as tile
    import concourse.mybir as mybir
    from concourse.vector_clock import ScopedClock

    if getattr(tile.TileContext, "_ctrl_wait_workaround", False):
        return

    def _drain_and_barrier(self, tick_clock, wait_clock):
        nc = self.nc
        drain_inst = nc.sync.drain()
        wait_clock.add_sem_waits(
            drain_inst.ins, ScopedClock({None: tick_clock.global_clock})
        )
        si = drain_inst.ins.sync_info
        waits = []
        if si is not None and si.on_wait:
            waits = list(si.on_wait)
            si.on_wait.clear()

        assert self.sems is not None
        popped = nc._tile_sem_poison_stack.pop()
        assert popped is self._sem_poison

        if waits:
            engines = [nc.vector, nc.scalar, nc.tensor, nc.sync, nc.gpsimd]
            join = nc.alloc_semaphore(name="tile-drain-join")
            for i, w in enumerate(waits):
                eng = engines[i % len(engines)]
                nop_inst = eng.nop()
                nop_inst.ins.sync_info = mybir.SyncInfo(on_wait=[w], on_update=[])
                nop_inst.then_inc(join, 1)
            jn = nc.gpsimd.nop()
            jn.wait_op(join, len(waits), "sem-ge")
            nc.clear_and_free_semaphores(list(self.sems.allocated().values()))
            nc.gpsimd.sem_clear(join)
            nc.release_semaphore(join)
        else:
            nc.clear_and_free_semaphores(list(self.sems.allocated().values()))

    tile.TileContext._drain_and_barrier = _drain_and_barrier
    tile.TileContext._ctrl_wait_workaround = True


def _split_multi_waits_json(bir_bytes):
    """The installed walrus accepts at most one sem-wait per instruction.
    Rewrite the serialized BIR: any instruction carrying N>1 waits keeps its
    last wait and gets N-1 single-wait NoOps inserted just before it on the
    same (in-order) engine queue."""
    import orjson

    bir = orjson.loads(bir_bytes)
    counter = [0]
    for fn in bir["functions"]:
        for blk in fn["blocks"]:
            new_insts = []
            for ins in blk["instructions"]:
                si = ins.get("sync_info")
                if si and len(si.get("on_wait") or []) > 1:
                    waits = si["on_wait"]
                    for w in waits[:-1]:
                        counter[0] += 1
                        new_insts.append({
                            "debug": ins.get("debug"),
                            "engine": ins["engine"],
                            "ins": [],
                            "name": f"I-waitsplit-{counter[0]}",
                            "opcode": "NoOp",
                            "outs": [],
                            "sync_info": {"on_update": [], "on_wait": [w]},
                        })
                    si["on_wait"] = [waits[-1]]
                new_insts.append(ins)
            blk["instructions"] = new_insts
    return orjson.dumps(bir)


def _build_program():
    _install_walrus_ctrl_wait_workaround()
    import concourse.bass as bass
    import concourse.tile as tile
    from concourse import mybir

    f32 = mybir.dt.float32
    bf16 = mybir.dt.bfloat16
    f16 = mybir.dt.float16
    mx = mybir.AluOpType.max

    nc = bass.Bass()
    # ga (cols 0:512) and pa (cols 512:1536) share one tensor so the fill's
    # critical path is a single DMA issue (the SP sequencer spends ~600ns
    # per DMA instruction).
    gp_d = nc.declare_dram_parameter(
        "gp", [B_LOC, 128, M + NP * 128], bf16, isOutput=False
    )
    d1p_d = nc.declare_dram_parameter(
        "d1p", [B_LOC, 128, T_OBJ, L3W], f16, isOutput=True
    )
    mc_d = nc.declare_dram_parameter("mc", [B_LOC, 128, 2, M], f16, isOutput=True)
    xg_d = nc.declare_dram_parameter("xg", [B_LOC, 2, 128, 4, M], f16, isOutput=True)

    with tile.TileContext(nc) as tc:
        with (
            tc.tile_pool(name="consts", bufs=1) as consts,
            tc.tile_pool(name="inputs", bufs=2) as inputs,
            tc.tile_pool(name="work", bufs=3) as work,
            tc.tile_pool(name="mm", bufs=2, space="PSUM") as mm_pool,
        ):
            # warm up ACT's Copy table while the first DMAs are in flight
            warm = consts.tile([1, 2], f32)
            nc.vector.memset(warm[:, 0:1], 0.0)
            nc.scalar.copy(out=warm[:, 1:2], in_=warm[:, 0:1])

            for b in range(B_LOC):
                # batch 0: one DMA covering ga + the first two packs' pa
                # columns (the minimum for the first PE pass), then the rest;
                # batch 1 queues behind as a single transfer.
                gp_sb = inputs.tile([128, M + NP * 128], bf16, tag="gp")
                if b == 0:
                    # first PE pass needs ga + pa cols 0:256: split across
                    # both HWDGE rings so the issues and transfers overlap
                    nc.sync.dma_start(out=gp_sb[:, 0:M], in_=gp_d[b, :, 0:M])
                    nc.scalar.dma_start(
                        out=gp_sb[:, M: M + 256], in_=gp_d[b, :, M: M + 256]
                    )
                    nc.sync.dma_start(out=gp_sb[:, M + 256:], in_=gp_d[b, :, M + 256:])
                else:
                    nc.sync.dma_start(out=gp_sb[:], in_=gp_d[b])

                macc2 = work.tile([128, 2, M], f16, tag="macc2")
                d1p = work.tile([128, T_OBJ, L3W], f16, tag="d1p")

                for g in range(NG):
                    x8 = work.tile([128, 8, M], f16, tag="x8")

                    # row path (dist1): 3 TT levels -> 64-wide partials for
                    # the first T_OBJ tiles only (obj rows packed first).
                    # Emitted after the first half-copy when it only needs
                    # those subtiles, so the d1p ship leaves the tail.
                    ntree = min(max(T_OBJ - 8 * g, 0), 8)

                    def emit_tree():
                        nc.vector.tensor_tensor(
                            out=d1p[:, 8 * g: 8 * g + ntree, :],
                            in0=x8[:, 0:ntree, 0:256],
                            in1=x8[:, 0:ntree, 256:512],
                            op=mx,
                        )
                        # ship row partials in two pieces so the bulk moves
                        # mid-kernel and only a sliver rides near the tail
                        done = 8 * g + ntree
                        if done == 16:
                            nc.sync.dma_start(
                                out=d1p_d[b, :, 0:16], in_=d1p[:, 0:16]
                            )
                        elif done >= T_OBJ:
                            nc.sync.dma_start(
                                out=d1p_d[b, :, 16:T_OBJ], in_=d1p[:, 16:T_OBJ]
                            )

                    for h in range(2):
                        p = 2 * g + h
                        grp = mm_pool.tile([128, PACK, M], f32, tag="grp")
                        for r in range(PACK):
                            nc.tensor.matmul(
                                grp[:, r, :],
                                gp_sb[
                                    32 * r: 32 * r + 20,
                                    M + p * 128: M + (p + 1) * 128,
                                ],
                                gp_sb[32 * r: 32 * r + 20, 0:M],
                                start=True,
                                stop=True,
                                tile_position=(32 * r, 0),
                            )
                        # PSUM egress, fp32 -> fp16. ACT is the bottleneck
                        # engine; DVE siphons tiles of one early pack per
                        # batch (2 of batch 0's pack 1, timed so the PSUM
                        # buffer is freed exactly when PE needs it; 1 of
                        # batch 1's pack 0 at the batch boundary). ACT's
                        # copy is emitted FIRST: readers of one PSUM tile
                        # are chained in emission order, so ACT must lead.
                        # DVE siphons egress only where its queue is empty
                        # (batch 0's fill, the batch boundary): mid-stream
                        # CASTs start late behind DVE's backlog, hold PSUM,
                        # and bubble ACT.
                        nv = 3 if (b == 0 and p == 1) or (b == 1 and p == 0) else 0
                        nc.scalar.copy(
                            out=x8[:, 4 * h + nv: 4 * h + 4, :],
                            in_=grp[:, nv:, :],
                        )
                        if nv:
                            nc.vector.tensor_copy(
                                out=x8[:, 4 * h: 4 * h + nv, :],
                                in_=grp[:, 0:nv, :],
                            )

                        # column path (dist2): pair-max the pack's 4 tiles,
                        # fold into the 2-plane accumulator
                        lo = x8[:, 4 * h: 4 * h + 2, :]
                        hi = x8[:, 4 * h + 2: 4 * h + 4, :]
                        if g < NG - 1:
                            if p == 0:
                                nc.vector.tensor_tensor(
                                    out=macc2[:], in0=lo, in1=hi, op=mx
                                )
                            else:
                                c1 = work.tile([128, 2, M], f16, tag="c1")
                                nc.vector.tensor_tensor(
                                    out=c1[:], in0=lo, in1=hi, op=mx
                                )
                                nc.vector.tensor_tensor(
                                    out=macc2[:], in0=macc2[:], in1=c1[:], op=mx
                                )
                        else:
                            # last group: ship the f16 tiles raw, straight
                            # off the ACT copy — no DVE op chains after the
                            # final PSUM egress; the host folds the planes.
                            # The final half splits across both HWDGE rings
                            # (DRAM completion latency ~2-3us dominates the
                            # tail).
                            if h == 0:
                                eng = nc.scalar if b == B_LOC - 1 else nc.sync
                                eng.dma_start(out=xg_d[b, 0], in_=x8[:, 0:4, :])
                            elif b < B_LOC - 1:
                                nc.sync.dma_start(out=xg_d[b, 1], in_=x8[:, 4:8, :])
                            else:
                                nc.sync.dma_start(
                                    out=xg_d[b, 1, :, 0:2], in_=x8[:, 4:6, :]
                                )
                                nc.scalar.dma_start(
                                    out=xg_d[b, 1, :, 2:4], in_=x8[:, 6:8, :]
                                )
                        if h == 0 and 0 < ntree <= 4 and b < B_LOC - 1:
                            emit_tree()

                    if g == NG - 2:
                        # macc2 final after this group's folds: ship before
                        # the final batch's small tree so the fold chain
                        # (and mc's DMA completion) finishes earliest
                        nc.sync.dma_start(out=mc_d[b], in_=macc2[:])
                    if ntree > 4 or (b == B_LOC - 1 and 0 < ntree <= 4):
                        emit_tree()

    _orig_to_json_bytes = nc.to_json_bytes
    nc.to_json_bytes = lambda: _split_multi_waits_json(_orig_to_json_bytes())
    return nc


def _get_program():
    if "nc" not in _PROGRAM_CACHE:
        _PROGRAM_CACHE["nc"] = _build_program()
    return _PROGRAM_CACHE["nc"]


def _hi_lo_split(x, bf16):
    hi = x.astype(bf16)
    lo = (x - hi.astype(np.float32)).astype(bf16)
    return hi, lo


def _prep_core_inputs(pred, gt, obj):
    """pred (B_LOC,N,3) gt (B_LOC,M,3) obj (B_LOC,N) int32.

    Rows with obj != 0 are permuted to the front per batch (row order is
    irrelevant to both reductions) so the kernel only row-reduces the first
    T_OBJ tiles. The matmul runs in bf16 with a hi/lo split (K=20): the four
    hi/lo row groups reproduce the fp32 dot products to ~2^-18 at bf16 PE
    speed. pa/ga are laid out for 4-way 32-row PE tiling: row group r
    (partitions 32r..32r+19) holds the K=20 rows; pa's group r carries pred
    tile 4p+r at columns p*128..p*128+127, ga is replicated into all four
    groups. Returns the DMA arrays plus the permuted obj (fp64) per batch."""
    import ml_dtypes
    bf16 = ml_dtypes.bfloat16

    pred = np.asarray(pred, np.float32)
    gt = np.asarray(gt, np.float32)
    obj = np.asarray(obj)

    pred_p = np.empty_like(pred)
    perm_obj = []
    for b in range(B_LOC):
        nz = obj[b] != 0
        k = int(nz.sum())
        assert k <= T_OBJ * 128, f"obj nonzero count {k} > capacity {T_OBJ * 128}"
        order = np.argsort(~nz, kind="stable")  # nonzero-obj rows first
        pred_p[b] = pred[b][order]
        perm_obj.append(np.asarray(obj[b], np.float64)[order])

    pa = np.empty((B_LOC, 5, N), np.float32)
    pa[:, 0:3] = -pred_p.transpose(0, 2, 1)
    pa[:, 3] = -np.square(pred_p).sum(-1)
    pa[:, 4] = -1.0
    ga = np.empty((B_LOC, 5, M), np.float32)
    ga[:, 0:3] = -2.0 * gt.transpose(0, 2, 1)
    ga[:, 3] = 1.0
    ga[:, 4] = np.square(gt).sum(-1)

    pa_hi, pa_lo = _hi_lo_split(pa, bf16)
    ga_hi, ga_lo = _hi_lo_split(ga, bf16)
    pa20 = np.concatenate([pa_hi, pa_hi, pa_lo, pa_lo], axis=1)  # [B_LOC, 20, N]
    ga20 = np.concatenate([ga_hi, ga_lo, ga_hi, ga_lo], axis=1)  # [B_LOC, 20, M]

    # gp[b, 32r+k, 0:M] = ga20[b, k];  gp[b, 32r+k, M + p*128+c] =
    # pa20[b, k, (p*PACK+r)*128 + c]
    gp = np.zeros((B_LOC, 128, M + NP * 128), bf16)
    pa_t = pa20.reshape(B_LOC, 20, NP, PACK, 128)
    for r in range(PACK):
        gp[:, 32 * r: 32 * r + 20, 0:M] = ga20
        gp[:, 32 * r: 32 * r + 20, M:] = (
            pa_t[:, :, :, r, :].reshape(B_LOC, 20, NP * 128)
        )

    return {"gp": gp}, perm_obj


def run(pred_center, center_label, box_label_mask, objectness_label, trace=False):
    """Run the sharded kernel; returns (loss_scalar, BassKernelResults)."""
    from concourse.bass_utils import run_bass_kernel_spmd

    nc = _get_program()
    in_maps = []
    perm_obj_all = []
    for c in range(N_CORES):
        bs = slice(B_LOC * c, B_LOC * (c + 1))
        m, pobj = _prep_core_inputs(
            pred_center[bs], center_label[bs], objectness_label[bs]
        )
        in_maps.append(m)
        perm_obj_all.append(pobj)
    res = run_bass_kernel_spmd(nc, in_maps, list(range(N_CORES)), trace=trace)

    mask = np.asarray(box_label_mask, np.float64)
    s1 = 0.0
    s2 = 0.0
    for c in range(N_CORES):
        r = res.results[c]
        d1p = np.asarray(r["d1p"], np.float32)  # [B_LOC, 128, T_OBJ, L3W]
        mc = np.asarray(r["mc"], np.float32)    # [B_LOC, 128, 2, M]
        xg = np.asarray(r["xg"], np.float32)    # [B_LOC, 2, 128, 4, M]
        for b in range(B_LOC):
            bi = B_LOC * c + b
            neg_d1 = d1p[b].reshape(128, T_OBJ, L3W).max(axis=2)  # [128, T_OBJ]
            dist1 = -neg_d1.T.reshape(-1).astype(np.float64)      # pred t*128+q
            s1 += float(dist1 @ perm_obj_all[c][b][: T_OBJ * 128])
            neg_d2 = np.maximum(
                mc[b].reshape(128, 2, M).max(axis=(0, 1)),
                xg[b].reshape(2 * 128 * 4, M).max(axis=0),
            )
            s2 += float((-neg_d2.astype(np.float64)) @ mask[bi])

    sum_obj = float(np.asarray(objectness_label, np.float64).sum())
    sum_mask = float(mask.sum())
    loss = s1 / (sum_obj + 1e-6) + s2 / (sum_mask + 1e-6)
    return np.float32(loss), res


def kernel(pred_center, center_label, box_label_mask, objectness_label):
    loss, _ = run(pred_center, center_label, box_label_mask, objectness_label)
    return np.array(loss, dtype=np.float32)


# revision 53
# speedup vs baseline: 1.0123x; 1.0123x over previous
"""Trainium2 Bass kernel for nn_CenterLoss (retrieval_knn).

reference semantics (per batch b):
    dist[n, m] = ||pred[b, n] - gt[b, m]||^2           (N=4096, M=512)
    dist1[n] = min_m dist ; dist2[m] = min_n dist
    loss = sum(dist1*obj)/(sum(obj)+1e-6) + sum(dist2*mask)/(sum(mask)+1e-6)

Strategy: data-parallel over batch (16 batches -> 8 cores, 2 each). On each
core, per batch, the PE builds the NEGATED distance matrix T = -dist via a
K=20 augmented matmul (bf16 hi/lo split reproduces fp32 dots to ~2^-18):
    T[i, j] = sum_k pa[k, i] * ga[k, j]
Since K=20 <= 32, four pred-tiles' matmuls are packed onto the PE at once
with 32-row tile_position groups (pa rows at partitions 32r..32r+19, ga
replicated in the same partition bands), quadrupling PE throughput.

The reduction pipeline is engineered around two hard TRN2 facts: (1) matmul
PSUM output must be fp32, and only ACT reads PSUM at line rate, so the
minimal egress is one ACT fp32->fp16 copy of every element (~29us/core,
and ACT runs it back-to-back with zero gaps); (2) DVE tensor_tensor max
runs at 2x only on f16 SBUF operands. Everything else is shaved so DVE
stays just under ACT (the measured multi-engine optimum):
  - obj rows are permuted to the front on the host; row-max trees (dist1)
    run on only T_OBJ=18 of 32 tiles (objectness_label is 0/1, so only
    ~2048 rows need dist1; capacity asserted host-side).
  - trees stop at 64-wide partials (3 TT levels); the host finishes the max.
  - the column path (dist2) folds pack-pair maxes into a 2-plane macc2
    accumulator over the first 3 groups; the last group's f16 tiles ship
    raw straight off the ACT copy, so NO engine op chains after the final
    PSUM egress — the kernel-exit critical path is just the last two DMAs
    (split across both HWDGE rings) and their DRAM completion.
  - DVE siphons ~5% of the PSUM egress (2 tiles of batch 0's pack 1 during
    the fill, 1 tile at the batch boundary) — the computed ACT/DVE balance
    point. Readers of one PSUM tile are chained in emission order, so the
    ACT copy is always emitted before the DVE cast of the same pack.
  - partials (d1p row partials, macc2, raw pair-maxes) DMA to DRAM; the
    host does the cross-partition max and both masked sums in fp64. No
    on-chip transposes, masked sums, or aux inputs at all.
  - ga+pa share one DRAM tensor (single head-DMA issue on the fill path);
    out-DMAs are spread across both HWDGE rings with the final pair-max
    alone on the SP ring (DRAM completion latency dominates the tail).
Teardown: the walrus single-wait limit forces the end-of-kernel drain to be
split into single-wait NOPs; these are round-robined across all five
engines (a serial GpSimd chain costs ~6us) with a join semaphore that
GpSimd waits on before clearing semaphores.
"""

import numpy as np

B, N, M = 16, 4096, 512
N_CORES = 8
B_LOC = B // N_CORES        # batches per core
NT = N // 128               # pred tiles per batch (32)
PACK = 4                    # pred tiles packed per PE pass (32-row groups)
NP = NT // PACK             # packs per batch (8)
NG = NP // 2                # x8 groups of 8 tiles (4)
T_OBJ = 18                  # row-tree tile capacity (obj!=0 rows first)
L3W = 256                   # shipped row-partial width per tile (1 TT level)

_PROGRAM_CACHE = {}


def _install_walrus_ctrl_wait_workaround():
    """The installed walrus rejects multi-wait CTRL (Drain) instructions
    ("Too many sync wait commands"). Split the TileContext end-of-kernel
    drain's sem waits onto individual NOPs (one wait each), round-robined
    across all five engines so they retire in parallel; a join semaphore
    orders GpSimd's semaphore clears after every wait."""
    import concourse.tile as tile
    import concourse.mybir as mybir
    from concourse.vector_clock import ScopedClock

    if getattr(tile.TileContext, "_ctrl_wait_workaround", False):
        return

    def _drain_and_barrier(self, tick_clock, wait_clock):
        nc = self.nc
        drain_inst = nc.sync.drain()
        wait_clock.add_sem_waits(
            drain_inst.ins, ScopedClock({None: tick_clock.global_clock})
        )
        si = drain_inst.ins.sync_info
        waits = []
        if si is not None and si.on_wait:
            waits = list(si.on_wait)
            si.on_wait.clear()

        assert self.sems is not None
        popped = nc._tile_sem_poison_stack.pop()
        assert popped is self._sem_poison

        if waits:
            engines = [nc.vector, nc.scalar, nc.tensor, nc.sync, nc.gpsimd]
            join = nc.alloc_semaphore(name="tile-drain-join")
            for i, w in enumerate(waits):
                eng = engines[i % len(engines)]
                nop_inst = eng.nop()
                nop_inst.ins.sync_info = mybir.SyncInfo(on_wait=[w], on_update=[])
                nop_inst.then_inc(join, 1)
            jn = nc.gpsimd.nop()
            jn.wait_op(join, len(waits), "sem-ge")
            nc.clear_and_free_semaphores(list(self.sems.allocated().values()))
            nc.gpsimd.sem_clear(join)
            nc.release_semaphore(join)
        else:
            nc.clear_and_free_semaphores(list(self.sems.allocated().values()))

    tile.TileContext._drain_and_barrier = _drain_and_barrier
    tile.TileContext._ctrl_wait_workaround = True


def _split_multi_waits_json(bir_bytes):
    """The installed walrus accepts at most one sem-wait per instruction.
    Rewrite the serialized BIR: any instruction carrying N>1 waits keeps its
    last wait and gets N-1 single-wait NoOps inserted just before it on the
    same (in-order) engine queue."""
    import orjson

    bir = orjson.loads(bir_bytes)
    counter = [0]
    for fn in bir["functions"]:
        for blk in fn["blocks"]:
            new_insts = []
            for ins in blk["instructions"]:
                si = ins.get("sync_info")
                if si and len(si.get("on_wait") or []) > 1:
                    waits = si["on_wait"]
                    for w in waits[:-1]:
                        counter[0] += 1
                        new_insts.append({
                            "debug": ins.get("debug"),
                            "engine": ins["engine"],
                            "ins": [],
                            "name": f"I-waitsplit-{counter[0]}",
                            "opcode": "NoOp",
                            "outs": [],
                            "sync_info": {"on_update": [], "on_wait": [w]},
                        })
                    si["on_wait"] = [waits[-1]]
                new_insts.append(ins)
            blk["instructions"] = new_insts
    return orjson.dumps(bir)


def _build_program():
    _install_walrus_ctrl_wait_workaround()
    import concourse.bass as bass
    import concourse.tile as tile
    from concourse import mybir

    f32 = mybir.dt.float32
    bf16 = mybir.dt.bfloat16
    f16 = mybir.dt.float16
    mx = mybir.AluOpType.max

    nc = bass.Bass()
    # ga (cols 0:512) and pa (cols 512:1536) share one tensor so the fill's
    # critical path is a single DMA issue (the SP sequencer spends ~600ns
    # per DMA instruction).
    gp_d = nc.declare_dram_parameter(
        "gp", [B_LOC, 128, M + NP * 128], bf16, isOutput=False
    )
    d1p_d = nc.declare_dram_parameter(
        "d1p", [B_LOC, 128, T_OBJ, L3W], f16, isOutput=True
    )
    mc_d = nc.declare_dram_parameter("mc", [B_LOC, 128, 2, M], f16, isOutput=True)
    xg_d = nc.declare_dram_parameter("xg", [B_LOC, 2, 128, 4, M], f16, isOutput=True)

    with tile.TileContext(nc) as tc:
        with (
            tc.tile_pool(name="consts", bufs=1) as consts,
            tc.tile_pool(name="inputs", bufs=2) as inputs,
            tc.tile_pool(name="work", bufs=3) as work,
            tc.tile_pool(name="mm", bufs=2, space="PSUM") as mm_pool,
        ):
            # warm up ACT's Copy table while the first DMAs are in flight
            warm = consts.tile([1, 2], f32)
            nc.vector.memset(warm[:, 0:1], 0.0)
            nc.scalar.copy(out=warm[:, 1:2], in_=warm[:, 0:1])

            for b in range(B_LOC):
                # batch 0: one DMA covering ga + the first two packs' pa
                # columns (the minimum for the first PE pass), then the rest;
                # batch 1 queues behind as a single transfer.
                gp_sb = inputs.tile([128, M + NP * 128], bf16, tag="gp")
                if b == 0:
                    # first PE pass needs ga + pa cols 0:256: split across
                    # both HWDGE rings so the issues and transfers overlap
                    nc.sync.dma_start(out=gp_sb[:, 0:M], in_=gp_d[b, :, 0:M])
                    nc.scalar.dma_start(
                        out=gp_sb[:, M: M + 256], in_=gp_d[b, :, M: M + 256]
                    )
                    nc.sync.dma_start(out=gp_sb[:, M + 256:], in_=gp_d[b, :, M + 256:])
                else:
                    nc.sync.dma_start(out=gp_sb[:], in_=gp_d[b])

                macc2 = work.tile([128, 2, M], f16, tag="macc2")
                d1p = work.tile([128, T_OBJ, L3W], f16, tag="d1p")

                for g in range(NG):
                    x8 = work.tile([128, 8, M], f16, tag="x8")

                    # row path (dist1): 3 TT levels -> 64-wide partials for
                    # the first T_OBJ tiles only (obj rows packed first).
                    # Emitted after the first half-copy when it only needs
                    # those subtiles, so the d1p ship leaves the tail.
                    ntree = min(max(T_OBJ - 8 * g, 0), 8)

                    def emit_tree():
                        nc.vector.tensor_tensor(
                            out=d1p[:, 8 * g: 8 * g + ntree, :],
                            in0=x8[:, 0:ntree, 0:256],
                            in1=x8[:, 0:ntree, 256:512],
                            op=mx,
                        )
                        # ship row partials in two pieces so the bulk moves
                        # mid-kernel and only a sliver rides near the tail
                        done = 8 * g + ntree
                        if done == 16:
                            nc.sync.dma_start(
                                out=d1p_d[b, :, 0:16], in_=d1p[:, 0:16]
                            )
                        elif done >= T_OBJ:
                            nc.sync.dma_start(
                                out=d1p_d[b, :, 16:T_OBJ], in_=d1p[:, 16:T_OBJ]
                            )

                    for h in range(2):
                        p = 2 * g + h
                        grp = mm_pool.tile([128, PACK, M], f32, tag="grp")
                        for r in range(PACK):
                            nc.tensor.matmul(
                                grp[:, r, :],
                                gp_sb[
                                    32 * r: 32 * r + 20,
                                    M + p * 128: M + (p + 1) * 128,
                                ],
                                gp_sb[32 * r: 32 * r + 20, 0:M],
                                start=True,
                                stop=True,
                                tile_position=(32 * r, 0),
                            )
                        # PSUM egress, fp32 -> fp16. ACT is the bottleneck
                        # engine; DVE siphons tiles of one early pack per
                        # batch (2 of batch 0's pack 1, timed so the PSUM
                        # buffer is freed exactly when PE needs it; 1 of
                        # batch 1's pack 0 at the batch boundary). ACT's
                        # copy is emitted FIRST: readers of one PSUM tile
                        # are chained in emission order, so ACT must lead.
                        # DVE siphons egress only where its queue is empty
                        # (batch 0's fill, the batch boundary): mid-stream
                        # CASTs start late behind DVE's backlog, hold PSUM,
                        # and bubble ACT.
                        nv = 2 if (b == 0 and p == 1) or (b == 1 and p == 0) else 0
                        nc.scalar.copy(
                            out=x8[:, 4 * h + nv: 4 * h + 4, :],
                            in_=grp[:, nv:, :],
                        )
                        if nv:
                            nc.vector.tensor_copy(
                                out=x8[:, 4 * h: 4 * h + nv, :],
                                in_=grp[:, 0:nv, :],
                            )

                        # column path (dist2): pair-max the pack's 4 tiles,
                        # fold into the 2-plane accumulator
                        lo = x8[:, 4 * h: 4 * h + 2, :]
                        hi = x8[:, 4 * h + 2: 4 * h + 4, :]
                        if g < NG - 1:
                            if p == 0:
                                nc.vector.tensor_tensor(
                                    out=macc2[:], in0=lo, in1=hi, op=mx
                                )
                            else:
                                c1 = work.tile([128, 2, M], f16, tag="c1")
                                nc.vector.tensor_tensor(
                                    out=c1[:], in0=lo, in1=hi, op=mx
                                )
                                nc.vector.tensor_tensor(
                                    out=macc2[:], in0=macc2[:], in1=c1[:], op=mx
                                )
                        else:
                            # last group: ship the f16 tiles raw, straight
                            # off the ACT copy — no DVE op chains after the
                            # final PSUM egress; the host folds the planes.
                            # The final half splits across both HWDGE rings
                            # (DRAM completion latency ~2-3us dominates the
                            # tail).
                            if h == 0:
                                eng = nc.scalar if b == B_LOC - 1 else nc.sync
                                eng.dma_start(out=xg_d[b, 0], in_=x8[:, 0:4, :])
                            elif b < B_LOC - 1:
                                nc.sync.dma_start(out=xg_d[b, 1], in_=x8[:, 4:8, :])
                            else:
                                nc.sync.dma_start(
                                    out=xg_d[b, 1, :, 0:2], in_=x8[:, 4:6, :]
                                )
                                nc.scalar.dma_start(
                                    out=xg_d[b, 1, :, 2:4], in_=x8[:, 6:8, :]
                                )
                        if h == 0 and 0 < ntree <= 4 and b < B_LOC - 1:
                            emit_tree()

                    if g == NG - 2:
                        # macc2 final after this group's folds: ship before
                        # the final batch's small tree so the fold chain
                        # (and mc's DMA completion) finishes earliest
                        nc.sync.dma_start(out=mc_d[b], in_=macc2[:])
                    if ntree > 4 or (b == B_LOC - 1 and 0 < ntree <= 4):
                        emit_tree()

    _orig_to_json_bytes = nc.to_json_bytes
    nc.to_json_bytes = lambda: _split_multi_waits_json(_orig_to_json_bytes())
    return nc


def _get_program():
    if "nc" not in _PROGRAM_CACHE:
        _PROGRAM_CACHE["nc"] = _build_program()
    return _PROGRAM_CACHE["nc"]


def _hi_lo_split(x, bf16):
    hi = x.astype(bf16)
    lo = (x - hi.astype(np.float32)).astype(bf16)
    return hi, lo


def _prep_core_inputs(pred, gt, obj):
    """pred (B_LOC,N,3) gt (B_LOC,M,3) obj (B_LOC,N) int32.

    Rows with obj != 0 are permuted to the front per batch (row order is
    irrelevant to both reductions) so the kernel only row-reduces the first
    T_OBJ tiles. The matmul runs in bf16 with a hi/lo split (K=20): the four
    hi/lo row groups reproduce the fp32 dot products to ~2^-18 at bf16 PE
    speed. pa/ga are laid out for 4-way 32-row PE tiling: row group r
    (partitions 32r..32r+19) holds the K=20 rows; pa's group r carries pred
    tile 4p+r at columns p*128..p*128+127, ga is replicated into all four
    groups. Returns the DMA arrays plus the permuted obj (fp64) per batch."""
    import ml_dtypes
    bf16 = ml_dtypes.bfloat16

    pred = np.asarray(pred, np.float32)
    gt = np.asarray(gt, np.float32)
    obj = np.asarray(obj)

    pred_p = np.empty_like(pred)
    perm_obj = []
    for b in range(B_LOC):
        nz = obj[b] != 0
        k = int(nz.sum())
        assert k <= T_OBJ * 128, f"obj nonzero count {k} > capacity {T_OBJ * 128}"
        order = np.argsort(~nz, kind="stable")  # nonzero-obj rows first
        pred_p[b] = pred[b][order]
        perm_obj.append(np.asarray(obj[b], np.float64)[order])

    pa = np.empty((B_LOC, 5, N), np.float32)
    pa[:, 0:3] = -pred_p.transpose(0, 2, 1)
    pa[:, 3] = -np.square(pred_p).sum(-1)
    pa[:, 4] = -1.0
    ga = np.empty((B_LOC, 5, M), np.float32)
    ga[:, 0:3] = -2.0 * gt.transpose(0, 2, 1)
    ga[:, 3] = 1.0
    ga[:, 4] = np.square(gt).sum(-1)

    pa_hi, pa_lo = _hi_lo_split(pa, bf16)
    ga_hi, ga_lo = _hi_lo_split(ga, bf16)
    pa20 = np.concatenate([pa_hi, pa_hi, pa_lo, pa_lo], axis=1)  # [B_LOC, 20, N]
    ga20 = np.concatenate([ga_hi, ga_lo, ga_hi, ga_lo], axis=1)  # [B_LOC, 20, M]

    # gp[b, 32r+k, 0:M] = ga20[b, k];  gp[b, 32r+k, M + p*128+c] =
    # pa20[b, k, (p*PACK+r)*128 + c]
    gp = np.zeros((B_LOC, 128, M + NP * 128), bf16)
    pa_t = pa20.reshape(B_LOC, 20, NP, PACK, 128)
    for r in range(PACK):
        gp[:, 32 * r: 32 * r + 20, 0:M] = ga20
        gp[:, 32 * r: 32 * r + 20, M:] = (
            pa_t[:, :, :, r, :].reshape(B_LOC, 20, NP * 128)
        )

    return {"gp": gp}, perm_obj


def run(pred_center, center_label, box_label_mask, objectness_label, trace=False):
    """Run the sharded kernel; returns (loss_scalar, BassKernelResults)."""
    from concourse.bass_utils import run_bass_kernel_spmd

    nc = _get_program()
    in_maps = []
    perm_obj_all = []
    for c in range(N_CORES):
        bs = slice(B_LOC * c, B_LOC * (c + 1))
        m, pobj = _prep_core_inputs(
            pred_center[bs], center_label[bs], objectness_label[bs]
        )
        in_maps.append(m)
        perm_obj_all.append(pobj)
    res = run_bass_kernel_spmd(nc, in_maps, list(range(N_CORES)), trace=trace)

    mask = np.asarray(box_label_mask, np.float64)
    s1 = 0.0
    s2 = 0.0
    for c in range(N_CORES):
        r = res.results[c]
        d1p = np.asarray(r["d1p"], np.float32)  # [B_LOC, 128, T_OBJ, L3W]
        mc = np.asarray(r["mc"], np.float32)    # [B_LOC, 128, 2, M]
        xg = np.asarray(r["xg"], np.float32)    # [B_LOC, 2, 128, 4, M]
        for b in range(B_LOC):
            bi = B_LOC * c + b
            neg_d1 = d1p[b].reshape(128, T_OBJ, L3W).max(axis=2)  # [128, T_OBJ]
            dist1 = -neg_d1.T.reshape(-1).astype(np.float64)      # pred t*128+q
            s1 += float(dist1 @ perm_obj_all[c][b][: T_OBJ * 128])
            neg_d2 = np.maximum(
                mc[b].reshape(128, 2, M).max(axis=(0, 1)),
                xg[b].reshape(2 * 128 * 4, M).max(axis=0),
            )
            s2 += float((-neg_d2.astype(np.float64)) @ mask[bi])

    sum_obj = float(np.asarray(objectness_label, np.float64).sum())
    sum_mask = float(mask.sum())
    loss = s1 / (sum_obj + 1e-6) + s2 / (sum_mask + 1e-6)
    return np.float32(loss), res


def kernel(pred_center, center_label, box_label_mask, objectness_label):
    loss, _ = run(pred_center, center_label, box_label_mask, objectness_label)
    return np.array(loss, dtype=np.float32)
